# revision 21
# baseline (speedup 1.0000x reference)
"""Trainium2 Bass kernel for nn_ConvolutionalAttention_3015067042131.

Math (reference.py):
  x [16,128,64,64] f32; x1 = x[:, :64], x2 = x[:, 64:]
  pooled = mean(x1, HW); h = gelu(pooled @ w1.T + b1); dyn = (h @ w2.T + b2) -> [B,64,9]
  x1_dyn = per-(batch,channel) 3x3 depthwise conv of x1 with dyn
  x1_lk  = conv2d(x1, lk_filter[64,64,13,13], SAME)
  out = concat([x1_lk + x1_dyn, x2], ch)

Strategy:
  * The tiny MLP (dyn) is computed on host in float64 (0.0007% of FLOPs).
  * The dynamic depthwise 3x3 is folded into the 13x13 conv weights as
    per-batch diagonal additions on the central 3x3 taps (3x3 tap (u,v)
    == 13x13 tap (u+5, v+5)); so the device runs ONE dense 13x13 conv
    with per-batch weights on 6 of 85 weight tiles.
  * Conv as shift-and-matmul: for each kernel tap, out[o, pix] +=
    W_tap[c, o].T @ xpad[c, pix+off]. Taps are paired along K=128:
    - xp: partitions 0-63 = zero-padded image (76x76), 64-127 = same
      shifted LEFT one column -> taps (i,j)+(i,j+1) fuse (78 tiles).
    - xq: partitions 0-63 = padded image, 64-127 = shifted UP one row
      -> column-12 taps pair as (i,12)+(i+1,12) (6 tiles) + 1 single.
    85 tiles cover all 169 taps (vs 91 with 13 half-empty singles).
  * Output pixels in 8 chunks of 512 (8 rows). Chunk pairs run
    CONCURRENTLY in the two PE column halves via tile_position (0,0) /
    (0,64) writing PSUM partitions 0-63 / 64-127 (measured 2x).
  * fp16 operands (HW fp16 matmul exact on rounded inputs, fp32 PSUM
    accumulate; end-to-end rel err ~4e-4). f32r broken in this stack,
    fp32 runs at 1/4 rate. Output stored f16 (halves out-DMA).
  * Head-latency tuning (trace-driven): input DMAs issue BEFORE the
    weight DMA (which is split in 3 so early tiles still arrive first);
    batch-0 image DMA is row-split so the on-chip pad-reshape (all on
    DVE; the scalar-engine copy was 3x slower and gated the baseline)
    starts earlier; PE warmup shortened to 8 matmuls so the ramp ends
    right as the first real matmul is ready.
  * Sharding: data-parallel over batch, 2 batches per core on 8 cores.
    x2 passthrough is host-side (no device work).
"""
import math

import numpy as np

B, C, H, W = 16, 128, 64, 64
PDIM, SK, LK = 64, 3, 13
PAD = LK // 2  # 6
HP, WP = H + 2 * PAD, W + 2 * PAD  # 76, 76
NCORES = 8
BPC = B // NCORES  # batches per core
NP = 85            # weight tiles: 78 row pairs + 6 col-12 pairs + 1 single
N_INTRA = 78       # t = i*6 + p -> pair ((i,2p),(i,2p+1))
NCHUNK = 8         # 512-pixel chunks per image
CHUNK = H * W // NCHUNK  # 512
WARMUP = 8
RS = 32            # batch-0 image DMA/copy row split

# central 3x3 taps (i,j in 5..7): j=5 -> B half of pair p=2; j=6,7 -> pair
# p=3; those 6 tiles are per-batch.
_MOD_TILES = [5 * 6 + 2, 6 * 6 + 2, 7 * 6 + 2, 5 * 6 + 3, 6 * 6 + 3, 7 * 6 + 3]
_MOD_SLOT = {t: s for s, t in enumerate(_MOD_TILES)}

_ERF = np.vectorize(math.erf, otypes=[np.float64])

_CACHED_NC = None


def _tap(t):
    """Tile t -> (iA, jA, src) with src 'xp' (col pair) or 'xq' (row pair)."""
    if t < N_INTRA:
        return t // 6, 2 * (t % 6), "xp"
    if t < N_INTRA + 6:
        return 2 * (t - N_INTRA), 12, "xq"
    return 12, 12, "xp"  # single (12,12), B half zero


def _build_nc():
    import concourse.mybir as mybir
    import concourse.tile as tile
    from concourse import bacc

    f32 = mybir.dt.float32
    f16 = mybir.dt.float16

    nc = bacc.Bacc(None, target_bir_lowering=False)
    xs = nc.dram_tensor("xs", [BPC, PDIM, H, W], f16, kind="ExternalInput")
    wsh = nc.dram_tensor("wsh", [128, NP * 64], f16, kind="ExternalInput")
    wmod = nc.dram_tensor("wmod", [BPC, 128, 6 * 64], f16, kind="ExternalInput")
    # chunk-major: row k = chunk*64 + out_channel
    y = nc.dram_tensor("y", [BPC, NCHUNK * PDIM, CHUNK], f16, kind="ExternalOutput")

    with tile.TileContext(nc) as tc:
        with (
            tc.tile_pool(name="wpool", bufs=1) as wpool,
            tc.tile_pool(name="wmpool", bufs=2) as wmpool,
            tc.tile_pool(name="xpool", bufs=2) as xpool,
            tc.tile_pool(name="opool", bufs=3) as opool,
            tc.tile_pool(name="pspool", bufs=4, space="PSUM") as pspool,
        ):
            wsh_sb = wpool.tile([128, NP * 64], f16)
            xst = [xpool.tile([128, H, W], f16, name=f"xst{b}") for b in range(BPC)]
            wm = [wmpool.tile([128, 6 * 64], f16, name=f"wm{b}") for b in range(BPC)]

            # -- DMA distribution: there are exactly TWO hardware DGE queues
            # (SP=sync and Activation=scalar); per-queue transfers serialize
            # in issue order, so each queue is a priority stream. Weight
            # pieces are sized so tile t lands before its matmul slot.
            def _w(eng, t0, t1):
                eng.dma_start(
                    out=wsh_sb[:, t0 * 64 : t1 * 64], in_=wsh[:, t0 * 64 : t1 * 64]
                )

            # sync queue: image A-half (top rows first), early weights, wm
            nc.sync.dma_start(out=xst[0][0:64, 0:RS, :], in_=xs[0, :, 0:RS, :])
            _w(nc.sync, 0, 8)
            _w(nc.sync, 22, 36)
            nc.sync.dma_start(out=wm[0][:], in_=wmod[0, :, :])
            nc.sync.dma_start(out=xst[0][0:64, RS:H, :], in_=xs[0, :, RS:H, :])
            nc.sync.dma_start(out=xst[1][0:64, :, :], in_=xs[1, :, :, :])
            nc.sync.dma_start(out=wm[1][:], in_=wmod[1, :, :])
            # scalar queue: image B-half, mid/late weights
            nc.scalar.dma_start(out=xst[0][64:128, 0:RS, :], in_=xs[0, :, 0:RS, :])
            _w(nc.scalar, 8, 22)
            _w(nc.scalar, 36, 64)
            nc.scalar.dma_start(out=xst[0][64:128, RS:H, :], in_=xs[0, :, RS:H, :])
            _w(nc.scalar, 64, NP)
            nc.scalar.dma_start(out=xst[1][64:128, :, :], in_=xs[1, :, :, :])

            # PE warmup: junk matmuls on a zeroed scratch tile so the HAM
            # un-throttles while the input DMAs + pad-reshape run.
            scratch = wpool.tile([128, CHUNK], f16)
            nc.vector.memset(scratch[:], 0.0)
            ps_warm = pspool.tile([128, CHUNK], f32, name="ps_warm", bufs=1)
            for wi in range(WARMUP):
                nc.tensor.matmul(
                    ps_warm[0:64, :],
                    lhsT=scratch[:, 0:64],
                    rhs=scratch[:, :],
                    start=(wi == 0),
                    stop=(wi == WARMUP - 1),
                    skip_group_check=True,
                )

            for b in range(BPC):
                st = xst[b]
                # xp: 0-63 padded image, 64-127 shifted left 1 col
                xp = xpool.tile([128, HP, WP], f16, name=f"xp{b}")
                nc.vector.memset(xp[:, 0:PAD, :], 0.0)
                nc.vector.memset(xp[:, PAD + H :, :], 0.0)
                nc.vector.memset(xp[0:64, PAD : PAD + H, 0:PAD], 0.0)
                nc.vector.memset(xp[0:64, PAD : PAD + H, PAD + W :], 0.0)
                nc.vector.memset(xp[64:128, PAD : PAD + H, 0 : PAD - 1], 0.0)
                nc.vector.memset(xp[64:128, PAD : PAD + H, PAD - 1 + W :], 0.0)
                if b == 0:
                    for r0, r1 in ((0, RS), (RS, H)):
                        nc.vector.tensor_copy(
                            xp[0:64, PAD + r0 : PAD + r1, PAD : PAD + W],
                            st[0:64, r0:r1, :],
                        )
                        nc.vector.tensor_copy(
                            xp[64:128, PAD + r0 : PAD + r1, PAD - 1 : PAD - 1 + W],
                            st[64:128, r0:r1, :],
                        )
                else:
                    nc.vector.tensor_copy(
                        xp[0:64, PAD : PAD + H, PAD : PAD + W], st[0:64, :, :]
                    )
                    nc.vector.tensor_copy(
                        xp[64:128, PAD : PAD + H, PAD - 1 : PAD - 1 + W],
                        st[64:128, :, :],
                    )
                # xq: 0-63 padded image, 64-127 shifted up 1 row (tiles 78-83,
                # ordered last in the accumulation so this may finish late)
                xq = xpool.tile([128, HP, WP], f16, name=f"xq{b}")
                nc.vector.memset(xq[:, 0 : PAD - 1, :], 0.0)
                nc.vector.memset(xq[0:64, PAD - 1 : PAD, :], 0.0)
                nc.vector.memset(xq[:, PAD + H :, :], 0.0)
                nc.vector.memset(xq[64:128, PAD - 1 + H : PAD + H, :], 0.0)
                nc.vector.memset(xq[0:64, PAD : PAD + H, 0:PAD], 0.0)
                nc.vector.memset(xq[0:64, PAD : PAD + H, PAD + W :], 0.0)
                nc.vector.memset(xq[64:128, PAD - 1 : PAD - 1 + H, 0:PAD], 0.0)
                nc.vector.memset(xq[64:128, PAD - 1 : PAD - 1 + H, PAD + W :], 0.0)
                nc.vector.tensor_copy(
                    xq[0:64, PAD : PAD + H, PAD : PAD + W], st[0:64, :, :]
                )
                nc.vector.tensor_copy(
                    xq[64:128, PAD - 1 : PAD - 1 + H, PAD : PAD + W], st[64:128, :, :]
                )

                for cp in range(NCHUNK // 2):
                    ps = pspool.tile([128, CHUNK], f32)
                    for t in range(NP):
                        s = _MOD_SLOT.get(t)
                        w_ap = (
                            wm[b][:, s * 64 : (s + 1) * 64]
                            if s is not None
                            else wsh_sb[:, t * 64 : (t + 1) * 64]
                        )
                        i, j, src = _tap(t)
                        xsrc = xq if src == "xq" else xp
                        for half in (0, 1):
                            r0 = i + 8 * (2 * cp + half)
                            nc.tensor.matmul(
                                ps[64 * half : 64 * (half + 1), :],
                                lhsT=w_ap,
                                rhs=xsrc[:, r0 : r0 + 8, j : j + 64],
                                start=(t == 0),
                                stop=(t == NP - 1),
                                tile_position=(0, 64 * half),
                                skip_group_check=True,
                            )
                    ot = opool.tile([128, CHUNK], f16)
                    nc.vector.tensor_copy(ot[:], ps[:])
                    nc.sync.dma_start(
                        out=y[b, 128 * cp : 128 * (cp + 1), :], in_=ot[:, :]
                    )
    nc.compile()
    return nc


def _get_nc():
    global _CACHED_NC
    if _CACHED_NC is None:
        _CACHED_NC = _build_nc()
    return _CACHED_NC


def _host_dyn(x, w1, b1, w2, b2):
    """dwc_proj MLP on host, float64: dyn [B, 64, 9]."""
    pooled = x[:, :PDIM].mean(axis=(2, 3), dtype=np.float64)      # [B, 64]
    z = pooled @ w1.T.astype(np.float64) + b1.astype(np.float64)  # [B, 32]
    h = 0.5 * z * (1.0 + _ERF(z / math.sqrt(2.0)))                # exact gelu
    dyn = h @ w2.T.astype(np.float64) + b2.astype(np.float64)     # [B, 576]
    return dyn.reshape(B, PDIM, SK * SK)


def _host_weights(lk_filter, dyn):
    """Build shared tap-pair weight tiles + per-batch modified central tiles.

    Weight tile t [128, 64]: rows 0-63 = lk[o, c, iA, jA].T (tap A), rows
    64-127 = tap B, in lhsT layout [K=c, M=o]. Tap B is (iA, jA+1) for the
    78 row-pair tiles, (iA+1, 12) for the 6 col-12 pair tiles, zero for the
    final single (12,12).
    """
    lkT = lk_filter.transpose(1, 0, 2, 3).astype(np.float32)  # [c, o, i, j]
    Wt = np.zeros((NP, 128, 64), np.float32)
    for t in range(NP):
        i, jA, src = _tap(t)
        Wt[t, 0:64, :] = lkT[:, :, i, jA]
        if src == "xp" and jA < 12:
            Wt[t, 64:128, :] = lkT[:, :, i, jA + 1]
        elif src == "xq":
            Wt[t, 64:128, :] = lkT[:, :, i + 1, jA]

    ar = np.arange(64)
    Wmod = np.zeros((B, 6, 128, 64), np.float32)
    for ii, i in enumerate((5, 6, 7)):
        t2, t3 = i * 6 + 2, i * 6 + 3
        u = i - 5
        for b in range(B):
            m2 = Wt[t2].copy()
            m3 = Wt[t3].copy()
            m2[64 + ar, ar] += dyn[b, :, u * 3 + 0].astype(np.float32)  # tap (i,5)
            m3[ar, ar] += dyn[b, :, u * 3 + 1].astype(np.float32)       # tap (i,6)
            m3[64 + ar, ar] += dyn[b, :, u * 3 + 2].astype(np.float32)  # tap (i,7)
            Wmod[b, ii] = m2
            Wmod[b, 3 + ii] = m3

    wsh_np = np.ascontiguousarray(
        Wt.transpose(1, 0, 2).reshape(128, NP * 64)
    ).astype(np.float16)
    wmod_np = np.ascontiguousarray(
        Wmod.transpose(0, 2, 1, 3).reshape(B, 128, 6 * 64)
    ).astype(np.float16)
    return wsh_np, wmod_np


def kernel(x, lk_filter, w1, b1, w2, b2):
    from concourse.bass_utils import run_bass_kernel_spmd

    x = np.asarray(x, dtype=np.float32)
    dyn = _host_dyn(x, np.asarray(w1), np.asarray(b1), np.asarray(w2), np.asarray(b2))
    wsh_np, wmod_np = _host_weights(np.asarray(lk_filter, dtype=np.float32), dyn)

    x1_f16 = x[:, :PDIM].astype(np.float16)  # [16, 64, 64, 64]

    nc = _get_nc()
    in_maps = []
    for k in range(NCORES):
        b0 = k * BPC
        in_maps.append(
            {
                "xs": np.ascontiguousarray(x1_f16[b0 : b0 + BPC]),
                "wsh": wsh_np,
                "wmod": np.ascontiguousarray(wmod_np[b0 : b0 + BPC]),
            }
        )
    res = run_bass_kernel_spmd(nc, in_maps, core_ids=list(range(NCORES)))

    out = np.empty((B, C, H, W), np.float32)
    for k in range(NCORES):
        b0 = k * BPC
        yk = res.results[k]["y"].reshape(BPC, NCHUNK, PDIM, CHUNK)
        out[b0 : b0 + BPC, :PDIM] = (
            yk.transpose(0, 2, 1, 3).reshape(BPC, PDIM, H, W).astype(np.float32)
        )
    out[:, PDIM:] = x[:, PDIM:]
    return out


# revision 22
# speedup vs baseline: 1.0106x; 1.0106x over previous
"""Trainium2 Bass kernel for nn_ConvolutionalAttention_3015067042131.

Math (reference.py):
  x [16,128,64,64] f32; x1 = x[:, :64], x2 = x[:, 64:]
  pooled = mean(x1, HW); h = gelu(pooled @ w1.T + b1); dyn = (h @ w2.T + b2) -> [B,64,9]
  x1_dyn = per-(batch,channel) 3x3 depthwise conv of x1 with dyn
  x1_lk  = conv2d(x1, lk_filter[64,64,13,13], SAME)
  out = concat([x1_lk + x1_dyn, x2], ch)

Strategy:
  * The tiny MLP (dyn) is computed on host in float64 (0.0007% of FLOPs).
  * The dynamic depthwise 3x3 is folded into the 13x13 conv weights as
    per-batch diagonal additions on the central 3x3 taps (3x3 tap (u,v)
    == 13x13 tap (u+5, v+5)); so the device runs ONE dense 13x13 conv
    with per-batch weights on 6 of 85 weight tiles.
  * Conv as shift-and-matmul: for each kernel tap, out[o, pix] +=
    W_tap[c, o].T @ xpad[c, pix+off]. Taps are paired along K=128:
    - xp: partitions 0-63 = zero-padded image (76x76), 64-127 = same
      shifted LEFT one column -> taps (i,j)+(i,j+1) fuse (78 tiles).
    - xq: partitions 0-63 = padded image, 64-127 = shifted UP one row
      -> column-12 taps pair as (i,12)+(i+1,12) (6 tiles) + 1 single.
    85 tiles cover all 169 taps (vs 91 with 13 half-empty singles).
  * Output pixels in 8 chunks of 512 (8 rows). Chunk pairs run
    CONCURRENTLY in the two PE column halves via tile_position (0,0) /
    (0,64) writing PSUM partitions 0-63 / 64-127 (measured 2x).
  * fp16 operands (HW fp16 matmul exact on rounded inputs, fp32 PSUM
    accumulate; end-to-end rel err ~4e-4). f32r broken in this stack,
    fp32 runs at 1/4 rate. Output stored f16 (halves out-DMA).
  * Head-latency tuning (trace-driven): input DMAs issue BEFORE the
    weight DMA (which is split in 3 so early tiles still arrive first);
    batch-0 image DMA is row-split so the on-chip pad-reshape (all on
    DVE; the scalar-engine copy was 3x slower and gated the baseline)
    starts earlier; PE warmup shortened to 8 matmuls so the ramp ends
    right as the first real matmul is ready.
  * Sharding: data-parallel over batch, 2 batches per core on 8 cores.
    x2 passthrough is host-side (no device work).
"""
import math

import numpy as np

B, C, H, W = 16, 128, 64, 64
PDIM, SK, LK = 64, 3, 13
PAD = LK // 2  # 6
HP, WP = H + 2 * PAD, W + 2 * PAD  # 76, 76
NCORES = 8
BPC = B // NCORES  # batches per core
NP = 85            # weight tiles: 78 row pairs + 6 col-12 pairs + 1 single
N_INTRA = 78       # t = i*6 + p -> pair ((i,2p),(i,2p+1))
NCHUNK = 8         # 512-pixel chunks per image
CHUNK = H * W // NCHUNK  # 512
WARMUP = 8
RS = 32            # batch-0 image DMA/copy row split

# central 3x3 taps (i,j in 5..7): j=5 -> B half of pair p=2; j=6,7 -> pair
# p=3; those 6 tiles are per-batch.
_MOD_TILES = [5 * 6 + 2, 6 * 6 + 2, 7 * 6 + 2, 5 * 6 + 3, 6 * 6 + 3, 7 * 6 + 3]
_MOD_SLOT = {t: s for s, t in enumerate(_MOD_TILES)}

_ERF = np.vectorize(math.erf, otypes=[np.float64])

_CACHED_NC = None


def _tap(t):
    """Tile t -> (iA, jA, src) with src 'xp' (col pair) or 'xq' (row pair)."""
    if t < N_INTRA:
        return t // 6, 2 * (t % 6), "xp"
    if t < N_INTRA + 6:
        return 2 * (t - N_INTRA), 12, "xq"
    return 12, 12, "xp"  # single (12,12), B half zero


def _build_nc():
    import concourse.mybir as mybir
    import concourse.tile as tile
    from concourse import bacc

    f32 = mybir.dt.float32
    f16 = mybir.dt.float16

    nc = bacc.Bacc(None, target_bir_lowering=False)
    xs = nc.dram_tensor("xs", [BPC, PDIM, H, W], f16, kind="ExternalInput")
    wsh = nc.dram_tensor("wsh", [128, NP * 64], f16, kind="ExternalInput")
    wmod = nc.dram_tensor("wmod", [BPC, 128, 6 * 64], f16, kind="ExternalInput")
    # chunk-major: row k = chunk*64 + out_channel
    y = nc.dram_tensor("y", [BPC, NCHUNK * PDIM, CHUNK], f16, kind="ExternalOutput")

    with tile.TileContext(nc) as tc:
        with (
            tc.tile_pool(name="wpool", bufs=1) as wpool,
            tc.tile_pool(name="wmpool", bufs=2) as wmpool,
            tc.tile_pool(name="xpool", bufs=2) as xpool,
            tc.tile_pool(name="opool", bufs=3) as opool,
            tc.tile_pool(name="pspool", bufs=4, space="PSUM") as pspool,
        ):
            wsh_sb = wpool.tile([128, NP * 64], f16)
            xst = [xpool.tile([128, H, W], f16, name=f"xst{b}") for b in range(BPC)]
            wm = [wmpool.tile([128, 6 * 64], f16, name=f"wm{b}") for b in range(BPC)]

            # -- DMA distribution: there are exactly TWO hardware DGE queues
            # (SP=sync and Activation=scalar); per-queue transfers serialize
            # in issue order, so each queue is a priority stream. Weight
            # pieces are sized so tile t lands before its matmul slot.
            def _w(eng, t0, t1):
                eng.dma_start(
                    out=wsh_sb[:, t0 * 64 : t1 * 64], in_=wsh[:, t0 * 64 : t1 * 64]
                )

            # sync queue: image A-half (top rows first), early weights, wm
            nc.sync.dma_start(out=xst[0][0:64, 0:RS, :], in_=xs[0, :, 0:RS, :])
            _w(nc.sync, 0, 8)
            _w(nc.sync, 22, 36)
            nc.sync.dma_start(out=wm[0][:], in_=wmod[0, :, :])
            nc.sync.dma_start(out=xst[0][0:64, RS:H, :], in_=xs[0, :, RS:H, :])
            nc.sync.dma_start(out=xst[1][0:64, :, :], in_=xs[1, :, :, :])
            nc.sync.dma_start(out=wm[1][:], in_=wmod[1, :, :])
            # scalar queue: image B-half, mid/late weights
            nc.scalar.dma_start(out=xst[0][64:128, 0:RS, :], in_=xs[0, :, 0:RS, :])
            _w(nc.scalar, 8, 22)
            _w(nc.scalar, 36, 64)
            nc.scalar.dma_start(out=xst[0][64:128, RS:H, :], in_=xs[0, :, RS:H, :])
            _w(nc.scalar, 64, NP)
            nc.scalar.dma_start(out=xst[1][64:128, :, :], in_=xs[1, :, :, :])

            # PE warmup: junk matmuls on a zeroed scratch tile so the HAM
            # un-throttles while the input DMAs + pad-reshape run.
            scratch = wpool.tile([128, CHUNK], f16)
            nc.vector.memset(scratch[:], 0.0)
            ps_warm = pspool.tile([128, CHUNK], f32, name="ps_warm", bufs=1)
            for wi in range(WARMUP):
                nc.tensor.matmul(
                    ps_warm[0:64, :],
                    lhsT=scratch[:, 0:64],
                    rhs=scratch[:, :],
                    start=(wi == 0),
                    stop=(wi == WARMUP - 1),
                    skip_group_check=True,
                )

            for b in range(BPC):
                st = xst[b]
                # xp: 0-63 padded image, 64-127 shifted left 1 col
                xp = xpool.tile([128, HP, WP], f16, name=f"xp{b}")
                nc.vector.memset(xp[:, 0:PAD, :], 0.0)
                nc.vector.memset(xp[:, PAD + H :, :], 0.0)
                nc.vector.memset(xp[0:64, PAD : PAD + H, 0:PAD], 0.0)
                nc.vector.memset(xp[0:64, PAD : PAD + H, PAD + W :], 0.0)
                nc.vector.memset(xp[64:128, PAD : PAD + H, 0 : PAD - 1], 0.0)
                nc.vector.memset(xp[64:128, PAD : PAD + H, PAD - 1 + W :], 0.0)
                if b == 0:
                    for r0, r1 in ((0, RS), (RS, H)):
                        nc.vector.tensor_copy(
                            xp[0:64, PAD + r0 : PAD + r1, PAD : PAD + W],
                            st[0:64, r0:r1, :],
                        )
                        nc.vector.tensor_copy(
                            xp[64:128, PAD + r0 : PAD + r1, PAD - 1 : PAD - 1 + W],
                            st[64:128, r0:r1, :],
                        )
                else:
                    nc.vector.tensor_copy(
                        xp[0:64, PAD : PAD + H, PAD : PAD + W], st[0:64, :, :]
                    )
                    nc.vector.tensor_copy(
                        xp[64:128, PAD : PAD + H, PAD - 1 : PAD - 1 + W],
                        st[64:128, :, :],
                    )
                # xq: 0-63 padded image, 64-127 shifted up 1 row (tiles 78-83,
                # ordered last in the accumulation so this may finish late)
                xq = xpool.tile([128, HP, WP], f16, name=f"xq{b}")
                nc.vector.memset(xq[:, 0 : PAD - 1, :], 0.0)
                nc.vector.memset(xq[0:64, PAD - 1 : PAD, :], 0.0)
                nc.vector.memset(xq[:, PAD + H :, :], 0.0)
                nc.vector.memset(xq[64:128, PAD - 1 + H : PAD + H, :], 0.0)
                nc.vector.memset(xq[0:64, PAD : PAD + H, 0:PAD], 0.0)
                nc.vector.memset(xq[0:64, PAD : PAD + H, PAD + W :], 0.0)
                nc.vector.memset(xq[64:128, PAD - 1 : PAD - 1 + H, 0:PAD], 0.0)
                nc.vector.memset(xq[64:128, PAD - 1 : PAD - 1 + H, PAD + W :], 0.0)
                nc.vector.tensor_copy(
                    xq[0:64, PAD : PAD + H, PAD : PAD + W], st[0:64, :, :]
                )
                nc.vector.tensor_copy(
                    xq[64:128, PAD - 1 : PAD - 1 + H, PAD : PAD + W], st[64:128, :, :]
                )

                for cp in range(NCHUNK // 2):
                    # The very last chunk-pair runs as two N=256 half-groups
                    # so the final eviction+DMA is half-sized and the
                    # penultimate one overlaps the remaining matmuls
                    # (shortens the post-last-matmul tail).
                    last = b == BPC - 1 and cp == NCHUNK // 2 - 1
                    halfgroups = 2 if last else 1
                    nsub = CHUNK // halfgroups
                    for hg in range(halfgroups):
                        if last:
                            ps = pspool.tile([128, nsub], f32, name="psh", bufs=2)
                        else:
                            ps = pspool.tile([128, CHUNK], f32, name="ps")
                        rows = 8 // halfgroups
                        for t in range(NP):
                            s = _MOD_SLOT.get(t)
                            w_ap = (
                                wm[b][:, s * 64 : (s + 1) * 64]
                                if s is not None
                                else wsh_sb[:, t * 64 : (t + 1) * 64]
                            )
                            i, j, src = _tap(t)
                            xsrc = xq if src == "xq" else xp
                            for half in (0, 1):
                                r0 = i + 8 * (2 * cp + half) + rows * hg
                                nc.tensor.matmul(
                                    ps[64 * half : 64 * (half + 1), :],
                                    lhsT=w_ap,
                                    rhs=xsrc[:, r0 : r0 + rows, j : j + 64],
                                    start=(t == 0),
                                    stop=(t == NP - 1),
                                    tile_position=(0, 64 * half),
                                    skip_group_check=True,
                                )
                        c0 = nsub * hg
                        ot = opool.tile([128, nsub], f16, name="ot", bufs=3)
                        nc.vector.tensor_copy(ot[:, :], ps[:, :])
                        # output rides both DGE queues, one partition half each
                        nc.sync.dma_start(
                            out=y[b, 128 * cp : 128 * cp + 64, c0 : c0 + nsub],
                            in_=ot[0:64, :],
                        )
                        nc.scalar.dma_start(
                            out=y[b, 128 * cp + 64 : 128 * (cp + 1), c0 : c0 + nsub],
                            in_=ot[64:128, :],
                        )
    nc.compile()
    return nc


def _get_nc():
    global _CACHED_NC
    if _CACHED_NC is None:
        _CACHED_NC = _build_nc()
    return _CACHED_NC


def _host_dyn(x, w1, b1, w2, b2):
    """dwc_proj MLP on host, float64: dyn [B, 64, 9]."""
    pooled = x[:, :PDIM].mean(axis=(2, 3), dtype=np.float64)      # [B, 64]
    z = pooled @ w1.T.astype(np.float64) + b1.astype(np.float64)  # [B, 32]
    h = 0.5 * z * (1.0 + _ERF(z / math.sqrt(2.0)))                # exact gelu
    dyn = h @ w2.T.astype(np.float64) + b2.astype(np.float64)     # [B, 576]
    return dyn.reshape(B, PDIM, SK * SK)


def _host_weights(lk_filter, dyn):
    """Build shared tap-pair weight tiles + per-batch modified central tiles.

    Weight tile t [128, 64]: rows 0-63 = lk[o, c, iA, jA].T (tap A), rows
    64-127 = tap B, in lhsT layout [K=c, M=o]. Tap B is (iA, jA+1) for the
    78 row-pair tiles, (iA+1, 12) for the 6 col-12 pair tiles, zero for the
    final single (12,12).
    """
    lkT = lk_filter.transpose(1, 0, 2, 3).astype(np.float32)  # [c, o, i, j]
    Wt = np.zeros((NP, 128, 64), np.float32)
    for t in range(NP):
        i, jA, src = _tap(t)
        Wt[t, 0:64, :] = lkT[:, :, i, jA]
        if src == "xp" and jA < 12:
            Wt[t, 64:128, :] = lkT[:, :, i, jA + 1]
        elif src == "xq":
            Wt[t, 64:128, :] = lkT[:, :, i + 1, jA]

    ar = np.arange(64)
    Wmod = np.zeros((B, 6, 128, 64), np.float32)
    for ii, i in enumerate((5, 6, 7)):
        t2, t3 = i * 6 + 2, i * 6 + 3
        u = i - 5
        for b in range(B):
            m2 = Wt[t2].copy()
            m3 = Wt[t3].copy()
            m2[64 + ar, ar] += dyn[b, :, u * 3 + 0].astype(np.float32)  # tap (i,5)
            m3[ar, ar] += dyn[b, :, u * 3 + 1].astype(np.float32)       # tap (i,6)
            m3[64 + ar, ar] += dyn[b, :, u * 3 + 2].astype(np.float32)  # tap (i,7)
            Wmod[b, ii] = m2
            Wmod[b, 3 + ii] = m3

    wsh_np = np.ascontiguousarray(
        Wt.transpose(1, 0, 2).reshape(128, NP * 64)
    ).astype(np.float16)
    wmod_np = np.ascontiguousarray(
        Wmod.transpose(0, 2, 1, 3).reshape(B, 128, 6 * 64)
    ).astype(np.float16)
    return wsh_np, wmod_np


def kernel(x, lk_filter, w1, b1, w2, b2):
    from concourse.bass_utils import run_bass_kernel_spmd

    x = np.asarray(x, dtype=np.float32)
    dyn = _host_dyn(x, np.asarray(w1), np.asarray(b1), np.asarray(w2), np.asarray(b2))
    wsh_np, wmod_np = _host_weights(np.asarray(lk_filter, dtype=np.float32), dyn)

    x1_f16 = x[:, :PDIM].astype(np.float16)  # [16, 64, 64, 64]

    nc = _get_nc()
    in_maps = []
    for k in range(NCORES):
        b0 = k * BPC
        in_maps.append(
            {
                "xs": np.ascontiguousarray(x1_f16[b0 : b0 + BPC]),
                "wsh": wsh_np,
                "wmod": np.ascontiguousarray(wmod_np[b0 : b0 + BPC]),
            }
        )
    res = run_bass_kernel_spmd(nc, in_maps, core_ids=list(range(NCORES)))

    out = np.empty((B, C, H, W), np.float32)
    for k in range(NCORES):
        b0 = k * BPC
        yk = res.results[k]["y"].reshape(BPC, NCHUNK, PDIM, CHUNK)
        out[b0 : b0 + BPC, :PDIM] = (
            yk.transpose(0, 2, 1, 3).reshape(BPC, PDIM, H, W).astype(np.float32)
        )
    out[:, PDIM:] = x[:, PDIM:]
    return out
